# revision 14
# baseline (speedup 1.0000x reference)
"""Trainium2 Bass kernel for CustomLSTM: B=64, T=1024, I=H=512.

Sharding: data-parallel over batch, 8 sequences per core on 8 cores.
Everything on-device lives in TRANSPOSED layout (hidden/gate dim on SBUF
partitions, batch on the free dim) so the per-step elementwise chain runs on
all 128 lanes and h^T feeds the next step's matmul directly, zero transposes.

The x@W+bias precompute is FUSED into the recurrence loop: each 128-step
macro iteration computes the xw needed two iterations later, using one
N=512 matmul per step and one bias-activation per 4 steps in the PE/ACT
idle slots of the step; a short prologue covers the first two iterations.

Per step: per-gate PSUM tiles get xw injected by an identity matmul
(start=True), then the 16 U matmuls accumulate on top (start=False), so
each sigmoid/tanh reads its PSUM tile directly after its gate's matmuls.
Gate order g,f,i,o overlaps the elementwise chain with the PE phase; h is
written as bf16 straight into the staging tile, which is also the h state
consumed by the next step's matmuls.
"""

import numpy as np
import ml_dtypes

B, T, I, H = 64, 1024, 512, 512
NC = 8            # cores
BL = B // NC      # 8 sequences per core
G4 = 4 * H        # 2048 gate dim
KT = I // 128     # 4 contraction tiles
MT = G4 // 128    # 16 gate m-tiles
C = T * BL        # 8192 columns, col = t*8 + b
MACRO = 128       # timesteps per For_i iteration
HM = MACRO // 2   # half-macro (xwm double-buffer granularity)
CHUNK = 64        # timesteps per precompute chunk (512 columns)
CCOL = CHUNK * BL  # 512 cols per chunk
XPAD = 3 * MACRO * BL   # x pad: last body prefetches 3 iterations ahead
WPAD = 2 * MACRO * BL   # xwT pad: fused precompute writes 2 iterations ahead

# m-tile gate map in W/U column order: i: 0-3, f: 4-7, g: 8-11, o: 12-15
GATE_M = {"i": 0, "f": 4, "g": 8, "o": 12}


def build(nc, bass, tile, mybir):
    f32, bf16 = mybir.dt.float32, mybir.dt.bfloat16
    AF = mybir.ActivationFunctionType

    xT = nc.dram_tensor("xT", [128, KT, C + XPAD], bf16, kind="ExternalInput")
    W = nc.dram_tensor("W", [128, KT, G4], bf16, kind="ExternalInput")
    U = nc.dram_tensor("U", [128, KT, G4], bf16, kind="ExternalInput")
    biasT = nc.dram_tensor("biasT", [128, MT], f32, kind="ExternalInput")
    ident = nc.dram_tensor("ident", [128, 128], bf16, kind="ExternalInput")
    hT_out = nc.dram_tensor("hT_out", [128, KT, C], bf16, kind="ExternalOutput")

    with tile.TileContext(nc) as tc:
        with (
            tc.tile_pool(name="const", bufs=1) as const,
            tc.tile_pool(name="state", bufs=1) as state,
            tc.tile_pool(name="work", bufs=2) as work,
            tc.tile_pool(name="dram", bufs=1, space="DRAM") as dram,
            tc.tile_pool(name="ps_g", bufs=1, space="PSUM") as ps_g_pool,
            tc.tile_pool(name="ps_f", bufs=1, space="PSUM") as ps_f_pool,
            tc.tile_pool(name="ps_i", bufs=1, space="PSUM") as ps_i_pool,
            tc.tile_pool(name="ps_o", bufs=1, space="PSUM") as ps_o_pool,
            tc.tile_pool(name="a_f", bufs=1, space="PSUM") as a_f_pool,
            tc.tile_pool(name="a_i", bufs=1, space="PSUM") as a_i_pool,
            tc.tile_pool(name="thc", bufs=1, space="PSUM") as thc_pool,
            tc.tile_pool(name="pre_ps", bufs=1, space="PSUM") as pre_ps_pool,
        ):
            W_sb = const.tile([128, KT, G4], bf16)
            U_sb = const.tile([128, KT, G4], bf16)
            bias_sb = const.tile([128, MT], f32)
            ident_sb = const.tile([128, 128], bf16)
            nc.gpsimd.dma_start(W_sb[:], W[:])
            nc.gpsimd.dma_start(U_sb[:], U[:])
            nc.gpsimd.dma_start(bias_sb[:], biasT[:])
            nc.gpsimd.dma_start(ident_sb[:], ident[:])

            # padded: fused precompute writes 2 iterations ahead
            xwT = dram.tile([128, MT, C + WPAD], bf16)

            GB = 4 * BL  # 32 cols per gate tile
            hT_st = state.tile([128, KT * BL], bf16)
            c_st = state.tile([128, KT * BL], f32)
            stage = state.tile([128, KT, MACRO * BL], bf16)
            xwmA = state.tile([128, MT, HM * BL], bf16)
            xwmB = state.tile([128, MT, HM * BL], bf16)
            xtcA = state.tile([128, KT, CCOL], bf16)
            xtcB = state.tile([128, KT, CCOL], bf16)
            xwcA = state.tile([128, MT, CCOL], bf16)
            xwcB = state.tile([128, MT, CCOL], bf16)

            pre_ps = pre_ps_pool.tile([128, CCOL], f32)
            ps_g = ps_g_pool.tile([128, GB], f32)
            ps_f = ps_f_pool.tile([128, GB], f32)
            ps_i = ps_i_pool.tile([128, GB], f32)
            ps_o = ps_o_pool.tile([128, GB], f32)
            PS = {"g": ps_g, "f": ps_f, "i": ps_i, "o": ps_o}

            def xw_act(xwc, m):
                nc.scalar.activation(
                    xwc[:, m, :], pre_ps[:], AF.Identity,
                    bias=bias_sb[:, m:m + 1],
                )

            # ---- Prologue: xw for cols [0, 2*MACRO*BL) the plain way ----
            for ch in range(4):
                xtc = xtcA if ch % 2 == 0 else xtcB
                xwc = xwcA if ch % 2 == 0 else xwcB
                cols = slice(ch * CCOL, (ch + 1) * CCOL)
                nc.gpsimd.dma_start(xtc[:], xT[:, :, cols])
                for m in range(MT):
                    for k in range(KT):
                        nc.tensor.matmul(
                            pre_ps[:],
                            W_sb[:, k, m * 128:(m + 1) * 128],
                            xtc[:, k, :],
                            start=(k == 0), stop=(k == KT - 1),
                        )
                    xw_act(xwc, m)
                nc.gpsimd.dma_start(xwT[:, :, cols], xwc[:])

            nc.vector.memset(hT_st[:], 0.0)
            nc.vector.memset(c_st[:], 0.0)
            nc.gpsimd.dma_start(xwmA[:], xwT[:, :, 0:HM * BL])
            # x for body 0's fused chunk 0 (cols 2 iterations ahead)
            nc.gpsimd.dma_start(
                xtcA[:], xT[:, :, 2 * MACRO * BL:2 * MACRO * BL + CCOL]
            )

            def mm_group(gate, h_prev, xwm, slot):
                m0 = GATE_M[gate]
                dst = PS[gate]
                # inject xw via identity matmul (start=True), then
                # accumulate the 16 U matmuls on top.
                nc.tensor.matmul(
                    dst[:].rearrange("p (m b) -> p m b", m=4),
                    ident_sb[:],
                    xwm[:, m0:m0 + 4, slot * BL:(slot + 1) * BL],
                    start=True,
                    stop=False,
                    skip_group_check=True,
                )
                for j in range(4):
                    m = m0 + j
                    for k in range(KT):
                        nc.tensor.matmul(
                            dst[:, j * BL:(j + 1) * BL],
                            U_sb[:, k, m * 128:(m + 1) * 128],
                            h_prev[:, k, :],
                            start=False,
                            stop=(k == KT - 1),
                            skip_group_check=True,
                        )

            def macro_body(c0, unroll):
                assert unroll == 1
                # prefetch second half of this iteration's xw; load x for
                # this body's fused chunk 1 (consumed from step HM on)
                nc.gpsimd.dma_start(
                    xwmB[:], xwT[:, :, bass.ds(c0 + HM * BL, HM * BL)]
                )
                nc.gpsimd.dma_start(
                    xtcB[:], xT[:, :, bass.ds(c0 + 2 * MACRO * BL + CCOL, CCOL)]
                )
                for s in range(MACRO):
                    if s == 0:
                        h_prev = hT_st[:].rearrange("p (k b) -> p k b", k=KT)
                    else:
                        h_prev = stage[:, :, (s - 1) * BL:s * BL]
                    xwm, slot = (xwmA, s) if s < HM else (xwmB, s - HM)
                    for gate in ("g", "f", "i", "o"):
                        mm_group(gate, h_prev, xwm, slot)
                    # fused precompute: one N=512 matmul per step
                    fch, fm, fk = s // CHUNK, (s % CHUNK) // KT, s % KT
                    fxtc = xtcA if fch == 0 else xtcB
                    nc.tensor.matmul(
                        pre_ps[:],
                        W_sb[:, fk, fm * 128:(fm + 1) * 128],
                        fxtc[:, fk, :],
                        start=(fk == 0), stop=(fk == KT - 1),
                    )

                    tg = work.tile([128, GB], f32, tag="tg")
                    so = work.tile([128, GB], f32, tag="so")
                    cf = work.tile([128, GB], f32, tag="cf")
                    ig = work.tile([128, GB], f32, tag="ig")
                    a_f = a_f_pool.tile([128, GB], f32)
                    a_i = a_i_pool.tile([128, GB], f32)
                    thc = thc_pool.tile([128, GB], f32)

                    nc.scalar.activation(tg[:], ps_g[:], AF.Tanh)
                    nc.scalar.activation(a_f[:], ps_f[:], AF.Sigmoid)
                    nc.vector.tensor_mul(cf[:], a_f[:], c_st[:])
                    nc.scalar.activation(a_i[:], ps_i[:], AF.Sigmoid)
                    nc.vector.tensor_mul(ig[:], a_i[:], tg[:])
                    nc.vector.tensor_add(c_st[:], cf[:], ig[:])
                    nc.scalar.activation(so[:], ps_o[:], AF.Sigmoid)
                    nc.scalar.activation(thc[:], c_st[:], AF.Tanh)
                    if s == MACRO - 1:
                        hdst = hT_st[:].rearrange("p (k b) -> p k b", k=KT)
                    else:
                        hdst = stage[:, :, s * BL:(s + 1) * BL]
                    nc.vector.tensor_mul(
                        hdst,
                        so[:].rearrange("p (k b) -> p k b", k=KT),
                        thc[:].rearrange("p (k b) -> p k b", k=KT),
                    )
                    # fused precompute: bias + cast after each m finishes
                    if fk == KT - 1:
                        xw_act(xwcA if fch == 0 else xwcB, fm)
                    if s == HM - 1:
                        # first half done: stage out, prefetch next
                        # iteration's first-half xw, store fused chunk 0
                        nc.gpsimd.dma_start(
                            hT_out[:, :, bass.ds(c0, HM * BL)],
                            stage[:, :, 0:HM * BL],
                        )
                        nc.gpsimd.dma_start(
                            xwT[:, :, bass.ds(c0 + 2 * MACRO * BL, CCOL)],
                            xwcA[:],
                        )
                        nc.gpsimd.dma_start(
                            xwmA[:],
                            xwT[:, :, bass.ds(c0 + MACRO * BL, HM * BL)],
                        )
                        # x for next body's fused chunk 0
                        nc.gpsimd.dma_start(
                            xtcA[:],
                            xT[:, :, bass.ds(c0 + 3 * MACRO * BL, CCOL)],
                        )
                    elif s == MACRO - 1:
                        nc.vector.tensor_copy(
                            stage[:, :, s * BL:(s + 1) * BL],
                            hT_st[:].rearrange("p (k b) -> p k b", k=KT),
                        )
                        nc.gpsimd.dma_start(
                            hT_out[:, :, bass.ds(c0 + HM * BL, HM * BL)],
                            stage[:, :, HM * BL:MACRO * BL],
                        )
                        nc.gpsimd.dma_start(
                            xwT[:, :, bass.ds(c0 + 2 * MACRO * BL + CCOL, CCOL)],
                            xwcB[:],
                        )

            tc.For_i_unrolled_general(
                start=0, end=C, step=MACRO * BL,
                unrollable_body=macro_body, max_unroll=1,
                hint_engines=(mybir.EngineType.PE,),
            )
    nc.finalize()
    return nc


def kernel(x, W, U, bias):
    import concourse.bass as bass
    import concourse.bacc as bacc
    import concourse.tile as tile
    import concourse.mybir as mybir
    from concourse.bass_utils import run_bass_kernel_spmd

    x = np.asarray(x, np.float32)
    W = np.asarray(W, np.float32)
    U = np.asarray(U, np.float32)
    bias = np.asarray(bias, np.float32)

    nc = build(bacc.Bacc("TRN2", target_bir_lowering=False, num_devices=NC), bass, tile, mybir)

    Wt = np.ascontiguousarray(W.reshape(KT, 128, G4).transpose(1, 0, 2)).astype(ml_dtypes.bfloat16)
    Ut = np.ascontiguousarray(
        U.reshape(KT, 128, G4).transpose(1, 0, 2)
    ).astype(ml_dtypes.bfloat16)
    bt = np.ascontiguousarray(bias.reshape(MT, 128).T)

    in_maps = []
    for i in range(NC):
        xl = x[i * BL:(i + 1) * BL]                     # [8, 1024, 512]
        xTl = np.ascontiguousarray(
            xl.transpose(2, 1, 0).reshape(KT, 128, C)   # [512, T, 8]->[4,128,C]
        ).transpose(1, 0, 2)                            # [128, 4, C]
        xTl = np.concatenate(
            [xTl, np.zeros((128, KT, XPAD), xTl.dtype)], axis=2
        )
        in_maps.append({
            "xT": np.ascontiguousarray(xTl).astype(ml_dtypes.bfloat16),
            "W": Wt, "U": Ut, "biasT": bt,
            "ident": np.eye(128, dtype=ml_dtypes.bfloat16),
        })

    import os
    trace = bool(os.environ.get("LSTM_TRACE"))
    res = run_bass_kernel_spmd(
        nc, in_maps, core_ids=list(range(NC)), trace=trace
    )
    if trace and res.exec_time_ns is not None:
        print(f"HW exec time: {res.exec_time_ns} ns")
        print("trace:", (res.instructions_and_trace or (None, None))[1])
    out = np.empty((B, T, H), np.float32)
    for i in range(NC):
        ho = np.asarray(res.results[i]["hT_out"]).astype(np.float32)  # [128, 4, C]
        out[i * BL:(i + 1) * BL] = (
            ho.reshape(128, KT, T, BL).transpose(3, 2, 1, 0).reshape(BL, T, H)
        )
    return out


# revision 15
# speedup vs baseline: 1.0175x; 1.0175x over previous
"""Trainium2 Bass kernel for CustomLSTM: B=64, T=1024, I=H=512.

Sharding: data-parallel over batch, 8 sequences per core on 8 cores.
Everything on-device lives in TRANSPOSED layout (hidden/gate dim on SBUF
partitions, batch on the free dim) so the per-step elementwise chain runs on
all 128 lanes and h^T feeds the next step's matmul directly, zero transposes.

The x@W+bias precompute is FUSED into the recurrence loop: each 128-step
macro iteration computes the xw needed two iterations later, using one
N=512 matmul per step and one bias-activation per 4 steps in the PE/ACT
idle slots of the step; a short prologue covers the first two iterations.

Per step: per-gate PSUM tiles get xw injected by an identity matmul
(start=True), then the 16 U matmuls accumulate on top (start=False), so
each sigmoid/tanh reads its PSUM tile directly after its gate's matmuls.
Gate order g,f,i,o overlaps the elementwise chain with the PE phase; h is
written as bf16 straight into the staging tile, which is also the h state
consumed by the next step's matmuls.
"""

import numpy as np
import ml_dtypes

B, T, I, H = 64, 1024, 512, 512
NC = 8            # cores
BL = B // NC      # 8 sequences per core
G4 = 4 * H        # 2048 gate dim
KT = I // 128     # 4 contraction tiles
MT = G4 // 128    # 16 gate m-tiles
C = T * BL        # 8192 columns, col = t*8 + b
MACRO = 128       # timesteps per For_i iteration
HM = MACRO // 2   # half-macro (xwm double-buffer granularity)
CHUNK = 64        # timesteps per precompute chunk (512 columns)
CCOL = CHUNK * BL  # 512 cols per chunk
XPAD = 3 * MACRO * BL   # x pad: last body prefetches 3 iterations ahead
WPAD = 2 * MACRO * BL   # xwT pad: fused precompute writes 2 iterations ahead

# m-tile gate map in W/U column order: i: 0-3, f: 4-7, g: 8-11, o: 12-15
GATE_M = {"i": 0, "f": 4, "g": 8, "o": 12}


def build(nc, bass, tile, mybir):
    f32, bf16 = mybir.dt.float32, mybir.dt.bfloat16
    AF = mybir.ActivationFunctionType

    xT = nc.dram_tensor("xT", [128, KT, C + XPAD], bf16, kind="ExternalInput")
    W = nc.dram_tensor("W", [128, KT, G4], bf16, kind="ExternalInput")
    U = nc.dram_tensor("U", [128, KT, G4], bf16, kind="ExternalInput")
    biasT = nc.dram_tensor("biasT", [128, MT], f32, kind="ExternalInput")
    ident = nc.dram_tensor("ident", [128, 128], bf16, kind="ExternalInput")
    hT_out = nc.dram_tensor("hT_out", [128, KT, C], bf16, kind="ExternalOutput")

    with tile.TileContext(nc) as tc:
        with (
            tc.tile_pool(name="const", bufs=1) as const,
            tc.tile_pool(name="state", bufs=1) as state,
            tc.tile_pool(name="work", bufs=2) as work,
            tc.tile_pool(name="dram", bufs=1, space="DRAM") as dram,
            tc.tile_pool(name="ps_g", bufs=1, space="PSUM") as ps_g_pool,
            tc.tile_pool(name="ps_f", bufs=1, space="PSUM") as ps_f_pool,
            tc.tile_pool(name="ps_i", bufs=1, space="PSUM") as ps_i_pool,
            tc.tile_pool(name="ps_o", bufs=1, space="PSUM") as ps_o_pool,
            tc.tile_pool(name="a_f", bufs=1, space="PSUM") as a_f_pool,
            tc.tile_pool(name="a_i", bufs=1, space="PSUM") as a_i_pool,
            tc.tile_pool(name="thc", bufs=1, space="PSUM") as thc_pool,
            tc.tile_pool(name="pre_ps", bufs=1, space="PSUM") as pre_ps_pool,
        ):
            W_sb = const.tile([128, KT, G4], bf16)
            U_sb = const.tile([128, KT, G4], bf16)
            bias_sb = const.tile([128, MT], f32)
            ident_sb = const.tile([128, 128], bf16)
            nc.gpsimd.dma_start(W_sb[:], W[:])
            nc.gpsimd.dma_start(U_sb[:], U[:])
            nc.gpsimd.dma_start(bias_sb[:], biasT[:])
            nc.gpsimd.dma_start(ident_sb[:], ident[:])

            # padded: fused precompute writes 2 iterations ahead
            xwT = dram.tile([128, MT, C + WPAD], bf16)

            GB = 4 * BL  # 32 cols per gate tile
            hT_st = state.tile([128, KT * BL], bf16)
            c_st = state.tile([128, KT * BL], f32)
            stage = state.tile([128, KT, MACRO * BL], bf16)
            xwmA = state.tile([128, MT, HM * BL], bf16)
            xwmB = state.tile([128, MT, HM * BL], bf16)
            xtcA = state.tile([128, KT, CCOL], bf16)
            xtcB = state.tile([128, KT, CCOL], bf16)
            xwcA = state.tile([128, MT, CCOL], bf16)
            xwcB = state.tile([128, MT, CCOL], bf16)

            pre_ps = pre_ps_pool.tile([128, CCOL], f32)
            ps_g = ps_g_pool.tile([128, GB], f32)
            ps_f = ps_f_pool.tile([128, GB], f32)
            ps_i = ps_i_pool.tile([128, GB], f32)
            ps_o = ps_o_pool.tile([128, GB], f32)
            PS = {"g": ps_g, "f": ps_f, "i": ps_i, "o": ps_o}

            def xw_act(xwc, m):
                nc.scalar.activation(
                    xwc[:, m, :], pre_ps[:], AF.Identity,
                    bias=bias_sb[:, m:m + 1],
                )

            # ---- Prologue: xw for cols [0, 2*MACRO*BL) the plain way ----
            for ch in range(4):
                xtc = xtcA if ch % 2 == 0 else xtcB
                xwc = xwcA if ch % 2 == 0 else xwcB
                cols = slice(ch * CCOL, (ch + 1) * CCOL)
                nc.gpsimd.dma_start(xtc[:], xT[:, :, cols])
                for m in range(MT):
                    for k in range(KT):
                        nc.tensor.matmul(
                            pre_ps[:],
                            W_sb[:, k, m * 128:(m + 1) * 128],
                            xtc[:, k, :],
                            start=(k == 0), stop=(k == KT - 1),
                        )
                    xw_act(xwc, m)
                nc.gpsimd.dma_start(xwT[:, :, cols], xwc[:])

            nc.vector.memset(hT_st[:], 0.0)
            nc.vector.memset(c_st[:], 0.0)
            nc.gpsimd.dma_start(xwmA[:], xwT[:, :, 0:HM * BL])
            # x for body 0's fused chunk 0 (cols 2 iterations ahead)
            nc.gpsimd.dma_start(
                xtcA[:], xT[:, :, 2 * MACRO * BL:2 * MACRO * BL + CCOL]
            )

            def mm_group(gate, h_prev, xwm, slot):
                m0 = GATE_M[gate]
                dst = PS[gate]
                # inject xw via identity matmul (start=True), then
                # accumulate the 16 U matmuls on top.
                nc.tensor.matmul(
                    dst[:].rearrange("p (m b) -> p m b", m=4),
                    ident_sb[:],
                    xwm[:, m0:m0 + 4, slot * BL:(slot + 1) * BL],
                    start=True,
                    stop=False,
                    skip_group_check=True,
                )
                for j in range(4):
                    m = m0 + j
                    for k in range(KT):
                        nc.tensor.matmul(
                            dst[:, j * BL:(j + 1) * BL],
                            U_sb[:, k, m * 128:(m + 1) * 128],
                            h_prev[:, k, :],
                            start=False,
                            stop=(k == KT - 1),
                            skip_group_check=True,
                        )

            def macro_body(c0, unroll):
                assert unroll == 1
                # prefetch second half of this iteration's xw; load x for
                # this body's fused chunk 1 (consumed from step HM on)
                nc.gpsimd.dma_start(
                    xwmB[:], xwT[:, :, bass.ds(c0 + HM * BL, HM * BL)]
                )
                nc.gpsimd.dma_start(
                    xtcB[:], xT[:, :, bass.ds(c0 + 2 * MACRO * BL + CCOL, CCOL)]
                )
                for s in range(MACRO):
                    if s == 0:
                        h_prev = hT_st[:].rearrange("p (k b) -> p k b", k=KT)
                    else:
                        h_prev = stage[:, :, (s - 1) * BL:s * BL]
                    xwm, slot = (xwmA, s) if s < HM else (xwmB, s - HM)
                    for gate in ("g", "f", "i", "o"):
                        mm_group(gate, h_prev, xwm, slot)
                    # fused precompute: one N=512 matmul per step
                    fch, fm, fk = s // CHUNK, (s % CHUNK) // KT, s % KT
                    fxtc = xtcA if fch == 0 else xtcB
                    nc.tensor.matmul(
                        pre_ps[:],
                        W_sb[:, fk, fm * 128:(fm + 1) * 128],
                        fxtc[:, fk, :],
                        start=(fk == 0), stop=(fk == KT - 1),
                    )

                    tg = work.tile([128, GB], f32, tag="tg")
                    so = work.tile([128, GB], f32, tag="so")
                    cf = work.tile([128, GB], f32, tag="cf")
                    ig = work.tile([128, GB], f32, tag="ig")
                    a_f = a_f_pool.tile([128, GB], f32)
                    a_i = a_i_pool.tile([128, GB], f32)
                    thc = thc_pool.tile([128, GB], f32)

                    nc.scalar.activation(tg[:], ps_g[:], AF.Tanh)
                    nc.scalar.activation(a_f[:], ps_f[:], AF.Sigmoid)
                    nc.vector.tensor_mul(cf[:], a_f[:], c_st[:])
                    nc.scalar.activation(a_i[:], ps_i[:], AF.Sigmoid)
                    nc.vector.tensor_mul(ig[:], a_i[:], tg[:])
                    nc.vector.tensor_add(c_st[:], cf[:], ig[:])
                    nc.scalar.activation(so[:], ps_o[:], AF.Sigmoid)
                    nc.scalar.activation(thc[:], c_st[:], AF.Tanh)
                    if s == MACRO - 1:
                        hdst = hT_st[:].rearrange("p (k b) -> p k b", k=KT)
                    else:
                        hdst = stage[:, :, s * BL:(s + 1) * BL]
                    nc.vector.tensor_mul(
                        hdst,
                        so[:].rearrange("p (k b) -> p k b", k=KT),
                        thc[:].rearrange("p (k b) -> p k b", k=KT),
                    )
                    # fused precompute: bias + cast after each m finishes,
                    # at LOW priority so the scheduler slots it in ACT's
                    # idle window after the chain, never before tanh(c)
                    if fk == KT - 1:
                        with tc.high_priority(-300):
                            xw_act(xwcA if fch == 0 else xwcB, fm)
                        # store finished quarter of xw (4 m-tiles)
                        if fm % 4 == 3:
                            fxwc = xwcA if fch == 0 else xwcB
                            nc.gpsimd.dma_start(
                                xwT[:, fm - 3:fm + 1,
                                    bass.ds(c0 + 2 * MACRO * BL + fch * CCOL,
                                            CCOL)],
                                fxwc[:, fm - 3:fm + 1, :],
                            )
                    if s == MACRO - 1:
                        nc.vector.tensor_copy(
                            stage[:, :, s * BL:(s + 1) * BL],
                            hT_st[:].rearrange("p (k b) -> p k b", k=KT),
                        )
                    if s % (MACRO // 4) == MACRO // 4 - 1:
                        # stage out finished quarter of h
                        q0 = (s + 1 - MACRO // 4) * BL
                        nc.gpsimd.dma_start(
                            hT_out[:, :, bass.ds(c0 + q0, MACRO // 4 * BL)],
                            stage[:, :, q0:q0 + MACRO // 4 * BL],
                        )
                    if s == HM + 7:
                        # prefetch next iteration's first-half xw
                        nc.gpsimd.dma_start(
                            xwmA[:],
                            xwT[:, :, bass.ds(c0 + MACRO * BL, HM * BL)],
                        )
                    if s == HM + 15:
                        # x for next body's fused chunk 0
                        nc.gpsimd.dma_start(
                            xtcA[:],
                            xT[:, :, bass.ds(c0 + 3 * MACRO * BL, CCOL)],
                        )

            tc.For_i_unrolled_general(
                start=0, end=C, step=MACRO * BL,
                unrollable_body=macro_body, max_unroll=1,
                hint_engines=(mybir.EngineType.PE,),
            )
    nc.finalize()
    return nc


def kernel(x, W, U, bias):
    import concourse.bass as bass
    import concourse.bacc as bacc
    import concourse.tile as tile
    import concourse.mybir as mybir
    from concourse.bass_utils import run_bass_kernel_spmd

    x = np.asarray(x, np.float32)
    W = np.asarray(W, np.float32)
    U = np.asarray(U, np.float32)
    bias = np.asarray(bias, np.float32)

    nc = build(bacc.Bacc("TRN2", target_bir_lowering=False, num_devices=NC), bass, tile, mybir)

    Wt = np.ascontiguousarray(W.reshape(KT, 128, G4).transpose(1, 0, 2)).astype(ml_dtypes.bfloat16)
    Ut = np.ascontiguousarray(
        U.reshape(KT, 128, G4).transpose(1, 0, 2)
    ).astype(ml_dtypes.bfloat16)
    bt = np.ascontiguousarray(bias.reshape(MT, 128).T)

    in_maps = []
    for i in range(NC):
        xl = x[i * BL:(i + 1) * BL]                     # [8, 1024, 512]
        xTl = np.ascontiguousarray(
            xl.transpose(2, 1, 0).reshape(KT, 128, C)   # [512, T, 8]->[4,128,C]
        ).transpose(1, 0, 2)                            # [128, 4, C]
        xTl = np.concatenate(
            [xTl, np.zeros((128, KT, XPAD), xTl.dtype)], axis=2
        )
        in_maps.append({
            "xT": np.ascontiguousarray(xTl).astype(ml_dtypes.bfloat16),
            "W": Wt, "U": Ut, "biasT": bt,
            "ident": np.eye(128, dtype=ml_dtypes.bfloat16),
        })

    import os
    trace = bool(os.environ.get("LSTM_TRACE"))
    res = run_bass_kernel_spmd(
        nc, in_maps, core_ids=list(range(NC)), trace=trace
    )
    if trace and res.exec_time_ns is not None:
        print(f"HW exec time: {res.exec_time_ns} ns")
        print("trace:", (res.instructions_and_trace or (None, None))[1])
    out = np.empty((B, T, H), np.float32)
    for i in range(NC):
        ho = np.asarray(res.results[i]["hT_out"]).astype(np.float32)  # [128, 4, C]
        out[i * BL:(i + 1) * BL] = (
            ho.reshape(128, KT, T, BL).transpose(3, 2, 1, 0).reshape(BL, T, H)
        )
    return out


# revision 19
# speedup vs baseline: 1.0836x; 1.0650x over previous
"""Trainium2 Bass kernel for CustomLSTM: B=64, T=1024, I=H=512.

Sharding: data-parallel over batch, 8 sequences per core on 8 cores.
Everything on-device lives in TRANSPOSED layout (hidden/gate dim on SBUF
partitions, batch on the free dim) so the per-step elementwise chain runs on
all 128 lanes and h^T feeds the next step's matmul directly, zero transposes.

The x@W+bias precompute is FUSED into the recurrence loop: each 128-step
macro iteration computes the xw needed two iterations later, using one
N=512 matmul per step and one bias-activation per 4 steps in the PE/ACT
idle slots of the step; a short prologue covers the first two iterations.

Per step: per-gate PSUM tiles get xw injected by an identity matmul
(start=True), then the 16 U matmuls accumulate on top (start=False), so
each sigmoid/tanh reads its PSUM tile directly after its gate's matmuls.
Gate order g,f,i,o overlaps the elementwise chain with the PE phase; h is
written as bf16 straight into the staging tile, which is also the h state
consumed by the next step's matmuls.
"""

import numpy as np
import ml_dtypes

B, T, I, H = 64, 1024, 512, 512
NC = 8            # cores
BL = B // NC      # 8 sequences per core
G4 = 4 * H        # 2048 gate dim
KT = I // 128     # 4 contraction tiles
MT = G4 // 128    # 16 gate m-tiles
C = T * BL        # 8192 columns, col = t*8 + b
MACRO = 128       # timesteps per For_i iteration
HM = MACRO // 2   # half-macro (xwm double-buffer granularity)
CHUNK = 64        # timesteps per precompute chunk (512 columns)
CCOL = CHUNK * BL  # 512 cols per chunk
XPAD = 3 * MACRO * BL   # x pad: last body prefetches 3 iterations ahead
WPAD = 2 * MACRO * BL   # xwT pad: fused precompute writes 2 iterations ahead

# m-tile gate map in W/U column order: i: 0-3, f: 4-7, g: 8-11, o: 12-15
GATE_M = {"i": 0, "f": 4, "g": 8, "o": 12}


def build(nc, bass, tile, mybir):
    f32, bf16 = mybir.dt.float32, mybir.dt.bfloat16
    AF = mybir.ActivationFunctionType

    xT = nc.dram_tensor("xT", [128, KT, C + XPAD], bf16, kind="ExternalInput")
    W = nc.dram_tensor("W", [128, KT, G4], bf16, kind="ExternalInput")
    U = nc.dram_tensor("U", [128, KT, G4], bf16, kind="ExternalInput")
    biasT = nc.dram_tensor("biasT", [128, MT], f32, kind="ExternalInput")
    ident = nc.dram_tensor("ident", [128, 128], bf16, kind="ExternalInput")
    hT_out = nc.dram_tensor("hT_out", [128, KT, C], bf16, kind="ExternalOutput")

    with tile.TileContext(nc) as tc:
        with (
            tc.tile_pool(name="const", bufs=1) as const,
            tc.tile_pool(name="state", bufs=1) as state,
            tc.tile_pool(name="work", bufs=2) as work,
            tc.tile_pool(name="dram", bufs=1, space="DRAM") as dram,
            tc.tile_pool(name="ps_g", bufs=1, space="PSUM") as ps_g_pool,
            tc.tile_pool(name="ps_f", bufs=1, space="PSUM") as ps_f_pool,
            tc.tile_pool(name="ps_i", bufs=1, space="PSUM") as ps_i_pool,
            tc.tile_pool(name="ps_o", bufs=1, space="PSUM") as ps_o_pool,
            tc.tile_pool(name="a_f", bufs=1, space="PSUM") as a_f_pool,
            tc.tile_pool(name="a_i", bufs=1, space="PSUM") as a_i_pool,
            tc.tile_pool(name="thc", bufs=1, space="PSUM") as thc_pool,
            tc.tile_pool(name="pre_ps", bufs=1, space="PSUM") as pre_ps_pool,
        ):
            W_sb = const.tile([128, KT, G4], bf16)
            U_sb = const.tile([128, KT, G4], bf16)
            bias_sb = const.tile([128, MT], f32)
            ident_sb = const.tile([128, 128], bf16)
            nc.gpsimd.dma_start(W_sb[:], W[:])
            nc.gpsimd.dma_start(U_sb[:], U[:])
            nc.gpsimd.dma_start(bias_sb[:], biasT[:])
            nc.gpsimd.dma_start(ident_sb[:], ident[:])

            # padded: fused precompute writes 2 iterations ahead
            xwT = dram.tile([128, MT, C + WPAD], bf16)

            GB = 4 * BL  # 32 cols per gate tile
            bias_step = state.tile([128, MT], f32)
            hT_st = state.tile([128, KT * BL], bf16)
            c_st = state.tile([128, KT * BL], f32)
            stage = state.tile([128, KT, MACRO * BL], bf16)
            xwmA = state.tile([128, MT, HM * BL], bf16)
            xwmB = state.tile([128, MT, HM * BL], bf16)
            xtcA = state.tile([128, KT, CCOL], bf16)
            xtcB = state.tile([128, KT, CCOL], bf16)
            xwcA = state.tile([128, MT, CCOL], bf16)
            xwcB = state.tile([128, MT, CCOL], bf16)

            pre_ps = pre_ps_pool.tile([128, CCOL], f32)
            ps_g = ps_g_pool.tile([128, GB], f32)
            ps_f = ps_f_pool.tile([128, GB], f32)
            ps_i = ps_i_pool.tile([128, GB], f32)
            ps_o = ps_o_pool.tile([128, GB], f32)
            PS = {"g": ps_g, "f": ps_f, "i": ps_i, "o": ps_o}

            def xw_act(xwc, m, bias_src=None):
                nc.scalar.activation(
                    xwc[:, m, :], pre_ps[:], AF.Identity,
                    bias=(bias_src if bias_src is not None
                          else bias_sb)[:, m:m + 1],
                )

            # ---- Prologue: xw for cols [0, 2*MACRO*BL) the plain way ----
            for ch in range(4):
                xtc = xtcA if ch % 2 == 0 else xtcB
                xwc = xwcA if ch % 2 == 0 else xwcB
                cols = slice(ch * CCOL, (ch + 1) * CCOL)
                nc.gpsimd.dma_start(xtc[:], xT[:, :, cols])
                for m in range(MT):
                    for k in range(KT):
                        nc.tensor.matmul(
                            pre_ps[:],
                            W_sb[:, k, m * 128:(m + 1) * 128],
                            xtc[:, k, :],
                            start=(k == 0), stop=(k == KT - 1),
                        )
                    xw_act(xwc, m)
                nc.gpsimd.dma_start(xwT[:, :, cols], xwc[:])

            nc.vector.memset(hT_st[:], 0.0)
            nc.vector.memset(c_st[:], 0.0)
            nc.gpsimd.dma_start(xwmA[:], xwT[:, :, 0:HM * BL])
            # x for body 0's fused chunk 0 (cols 2 iterations ahead)
            nc.gpsimd.dma_start(
                xtcA[:], xT[:, :, 2 * MACRO * BL:2 * MACRO * BL + CCOL]
            )

            def mm_group(gate, h_prev, xwm, slot):
                m0 = GATE_M[gate]
                dst = PS[gate]
                # inject xw via identity matmul (start=True), then
                # accumulate the 16 U matmuls on top.
                nc.tensor.matmul(
                    dst[:].rearrange("p (m b) -> p m b", m=4),
                    ident_sb[:],
                    xwm[:, m0:m0 + 4, slot * BL:(slot + 1) * BL],
                    start=True,
                    stop=False,
                    skip_group_check=True,
                )
                for j in range(4):
                    m = m0 + j
                    for k in range(KT):
                        nc.tensor.matmul(
                            dst[:, j * BL:(j + 1) * BL],
                            U_sb[:, k, m * 128:(m + 1) * 128],
                            h_prev[:, k, :],
                            start=False,
                            stop=(k == KT - 1),
                            skip_group_check=True,
                        )

            def macro_body(c0, unroll):
                assert unroll == 1
                # prefetch second half of this iteration's xw; load x for
                # this body's fused chunk 1 (consumed from step HM on)
                nc.gpsimd.dma_start(
                    xwmB[:], xwT[:, :, bass.ds(c0 + HM * BL, HM * BL)]
                )
                nc.gpsimd.dma_start(
                    xtcB[:], xT[:, :, bass.ds(c0 + 2 * MACRO * BL + CCOL, CCOL)]
                )
                for s in range(MACRO):
                    if s == 0:
                        h_prev = hT_st[:].rearrange("p (k b) -> p k b", k=KT)
                    else:
                        h_prev = stage[:, :, (s - 1) * BL:s * BL]
                    xwm, slot = (xwmA, s) if s < HM else (xwmB, s - HM)
                    for gate in ("g", "f", "i", "o"):
                        mm_group(gate, h_prev, xwm, slot)
                    # fused precompute: one N=512 matmul per step
                    fch, fm, fk = s // CHUNK, (s % CHUNK) // KT, s % KT
                    fxtc = xtcA if fch == 0 else xtcB
                    nc.tensor.matmul(
                        pre_ps[:],
                        W_sb[:, fk, fm * 128:(fm + 1) * 128],
                        fxtc[:, fk, :],
                        start=(fk == 0), stop=(fk == KT - 1),
                    )

                    tg = work.tile([128, GB], f32, tag="tg")
                    so = work.tile([128, GB], f32, tag="so")
                    cf = work.tile([128, GB], f32, tag="cf")
                    ig = work.tile([128, GB], f32, tag="ig")
                    a_f = a_f_pool.tile([128, GB], f32)
                    a_i = a_i_pool.tile([128, GB], f32)
                    thc = thc_pool.tile([128, GB], f32)

                    nc.scalar.activation(tg[:], ps_g[:], AF.Tanh)
                    nc.scalar.activation(a_f[:], ps_f[:], AF.Sigmoid)
                    nc.vector.tensor_mul(cf[:], a_f[:], c_st[:])
                    nc.scalar.activation(a_i[:], ps_i[:], AF.Sigmoid)
                    nc.vector.tensor_mul(ig[:], a_i[:], tg[:])
                    nc.vector.tensor_add(c_st[:], cf[:], ig[:])
                    nc.scalar.activation(so[:], ps_o[:], AF.Sigmoid)
                    nc.scalar.activation(thc[:], c_st[:], AF.Tanh)
                    if s == MACRO - 1:
                        hdst = hT_st[:].rearrange("p (k b) -> p k b", k=KT)
                    else:
                        hdst = stage[:, :, s * BL:(s + 1) * BL]
                    nc.vector.tensor_mul(
                        hdst,
                        so[:].rearrange("p (k b) -> p k b", k=KT),
                        thc[:].rearrange("p (k b) -> p k b", k=KT),
                    )
                    # fused precompute: bias + cast after each m finishes.
                    # bias is read through bias_step, refreshed by DVE right
                    # after hmul: a real dependency that stops the scheduler
                    # from running this 679ns ACT op before tanh(c).
                    if fk == KT - 1:
                        nc.vector.scalar_tensor_tensor(
                            bias_step[:], thc[:, 0:MT], 0.0, bias_sb[:],
                            op0=mybir.AluOpType.mult,
                            op1=mybir.AluOpType.add,
                        )
                        xw_act(xwcA if fch == 0 else xwcB, fm,
                               bias_src=bias_step)
                        # store finished quarter of xw (4 m-tiles)
                        if fm % 4 == 3:
                            fxwc = xwcA if fch == 0 else xwcB
                            nc.gpsimd.dma_start(
                                xwT[:, fm - 3:fm + 1,
                                    bass.ds(c0 + 2 * MACRO * BL + fch * CCOL,
                                            CCOL)],
                                fxwc[:, fm - 3:fm + 1, :],
                            )
                    if s == MACRO - 1:
                        nc.vector.tensor_copy(
                            stage[:, :, s * BL:(s + 1) * BL],
                            hT_st[:].rearrange("p (k b) -> p k b", k=KT),
                        )
                    if s % (MACRO // 4) == MACRO // 4 - 1:
                        # stage out finished quarter of h
                        q0 = (s + 1 - MACRO // 4) * BL
                        nc.gpsimd.dma_start(
                            hT_out[:, :, bass.ds(c0 + q0, MACRO // 4 * BL)],
                            stage[:, :, q0:q0 + MACRO // 4 * BL],
                        )
                    if s == HM + 7:
                        # prefetch next iteration's first-half xw
                        nc.gpsimd.dma_start(
                            xwmA[:],
                            xwT[:, :, bass.ds(c0 + MACRO * BL, HM * BL)],
                        )
                    if s == HM + 15:
                        # x for next body's fused chunk 0
                        nc.gpsimd.dma_start(
                            xtcA[:],
                            xT[:, :, bass.ds(c0 + 3 * MACRO * BL, CCOL)],
                        )

            tc.For_i_unrolled_general(
                start=0, end=C, step=MACRO * BL,
                unrollable_body=macro_body, max_unroll=1,
                hint_engines=(mybir.EngineType.PE,),
            )
    nc.finalize()
    return nc


def kernel(x, W, U, bias):
    import concourse.bass as bass
    import concourse.bacc as bacc
    import concourse.tile as tile
    import concourse.mybir as mybir
    from concourse.bass_utils import run_bass_kernel_spmd

    x = np.asarray(x, np.float32)
    W = np.asarray(W, np.float32)
    U = np.asarray(U, np.float32)
    bias = np.asarray(bias, np.float32)

    nc = build(bacc.Bacc("TRN2", target_bir_lowering=False, num_devices=NC), bass, tile, mybir)

    Wt = np.ascontiguousarray(W.reshape(KT, 128, G4).transpose(1, 0, 2)).astype(ml_dtypes.bfloat16)
    Ut = np.ascontiguousarray(
        U.reshape(KT, 128, G4).transpose(1, 0, 2)
    ).astype(ml_dtypes.bfloat16)
    bt = np.ascontiguousarray(bias.reshape(MT, 128).T)

    in_maps = []
    for i in range(NC):
        xl = x[i * BL:(i + 1) * BL]                     # [8, 1024, 512]
        xTl = np.ascontiguousarray(
            xl.transpose(2, 1, 0).reshape(KT, 128, C)   # [512, T, 8]->[4,128,C]
        ).transpose(1, 0, 2)                            # [128, 4, C]
        xTl = np.concatenate(
            [xTl, np.zeros((128, KT, XPAD), xTl.dtype)], axis=2
        )
        in_maps.append({
            "xT": np.ascontiguousarray(xTl).astype(ml_dtypes.bfloat16),
            "W": Wt, "U": Ut, "biasT": bt,
            "ident": np.eye(128, dtype=ml_dtypes.bfloat16),
        })

    import os
    trace = bool(os.environ.get("LSTM_TRACE"))
    res = run_bass_kernel_spmd(
        nc, in_maps, core_ids=list(range(NC)), trace=trace
    )
    if trace and res.exec_time_ns is not None:
        print(f"HW exec time: {res.exec_time_ns} ns")
        print("trace:", (res.instructions_and_trace or (None, None))[1])
    out = np.empty((B, T, H), np.float32)
    for i in range(NC):
        ho = np.asarray(res.results[i]["hT_out"]).astype(np.float32)  # [128, 4, C]
        out[i * BL:(i + 1) * BL] = (
            ho.reshape(128, KT, T, BL).transpose(3, 2, 1, 0).reshape(BL, T, H)
        )
    return out


# revision 26
# speedup vs baseline: 1.0963x; 1.0117x over previous
"""Trainium2 Bass kernel for CustomLSTM: B=64, T=1024, I=H=512.

Sharding: data-parallel over batch, 8 sequences per core on 8 cores.
Everything on-device lives in TRANSPOSED layout (hidden/gate dim on SBUF
partitions, batch on the free dim) so the per-step elementwise chain runs on
all 128 lanes and h^T feeds the next step's matmul directly, zero transposes.

The x@W+bias precompute is FUSED into the recurrence loop: each 128-step
macro iteration computes the xw needed two iterations later, using one
N=512 matmul per step and one bias-activation per 4 steps in the PE/ACT
idle slots of the step; a short prologue covers the first two iterations.

Per step: per-gate PSUM tiles get xw injected by an identity matmul
(start=True), then the 16 U matmuls accumulate on top (start=False), so
each sigmoid/tanh reads its PSUM tile directly after its gate's matmuls.
Gate order g,f,i,o overlaps the elementwise chain with the PE phase; h is
written as bf16 straight into the staging tile, which is also the h state
consumed by the next step's matmuls.
"""

import numpy as np
import ml_dtypes

B, T, I, H = 64, 1024, 512, 512
NC = 8            # cores
BL = B // NC      # 8 sequences per core
G4 = 4 * H        # 2048 gate dim
KT = I // 128     # 4 contraction tiles
MT = G4 // 128    # 16 gate m-tiles
C = T * BL        # 8192 columns, col = t*8 + b
MACRO = 256       # timesteps per For_i iteration
HM = MACRO // 2   # half-macro (xwm double-buffer granularity)
CHUNK = 64        # timesteps per precompute chunk (512 columns)
CCOL = CHUNK * BL  # 512 cols per chunk
XPAD = 3072       # x pad: last body's fused reads + prefetches run ahead
WPAD = 2048       # xwT pad: fused precompute writes 1 iteration ahead

# m-tile gate map in W/U column order: i: 0-3, f: 4-7, g: 8-11, o: 12-15
GATE_M = {"i": 0, "f": 4, "g": 8, "o": 12}


def build(nc, bass, tile, mybir):
    f32, bf16 = mybir.dt.float32, mybir.dt.bfloat16
    AF = mybir.ActivationFunctionType

    xT = nc.dram_tensor("xT", [128, KT, C + XPAD], bf16, kind="ExternalInput")
    W = nc.dram_tensor("W", [128, KT, G4], bf16, kind="ExternalInput")
    U = nc.dram_tensor("U", [128, KT, G4], bf16, kind="ExternalInput")
    biasT = nc.dram_tensor("biasT", [128, MT], f32, kind="ExternalInput")
    ident = nc.dram_tensor("ident", [128, 128], bf16, kind="ExternalInput")
    hT_out = nc.dram_tensor("hT_out", [128, KT, C], bf16, kind="ExternalOutput")

    with tile.TileContext(nc) as tc:
        with (
            tc.tile_pool(name="const", bufs=1) as const,
            tc.tile_pool(name="state", bufs=1) as state,
            tc.tile_pool(name="work", bufs=2) as work,
            tc.tile_pool(name="dram", bufs=1, space="DRAM") as dram,
            tc.tile_pool(name="ps_g", bufs=1, space="PSUM") as ps_g_pool,
            tc.tile_pool(name="ps_f", bufs=1, space="PSUM") as ps_f_pool,
            tc.tile_pool(name="ps_i", bufs=1, space="PSUM") as ps_i_pool,
            tc.tile_pool(name="ps_o", bufs=1, space="PSUM") as ps_o_pool,
            tc.tile_pool(name="a_f", bufs=1, space="PSUM") as a_f_pool,
            tc.tile_pool(name="a_i", bufs=1, space="PSUM") as a_i_pool,
            tc.tile_pool(name="thc", bufs=1, space="PSUM") as thc_pool,
            tc.tile_pool(name="pre_ps", bufs=1, space="PSUM") as pre_ps_pool,
        ):
            W_sb = const.tile([128, KT, G4], bf16)
            U_sb = const.tile([128, KT, G4], bf16)
            bias_sb = const.tile([128, MT], f32)
            ident_sb = const.tile([128, 128], bf16)
            nc.gpsimd.dma_start(W_sb[:], W[:])
            nc.gpsimd.dma_start(U_sb[:], U[:])
            nc.gpsimd.dma_start(bias_sb[:], biasT[:])
            nc.gpsimd.dma_start(ident_sb[:], ident[:])

            # padded: fused precompute writes 2 iterations ahead
            xwT = dram.tile([128, MT, C + WPAD], bf16)

            GB = 4 * BL  # 32 cols per gate tile
            bias_step = state.tile([128, MT], f32)
            hT_st = state.tile([128, KT * BL], bf16)
            c_st = state.tile([128, KT * BL], f32)
            stage = state.tile([128, KT, MACRO * BL], bf16)
            xwmA = state.tile([128, MT, HM * BL], bf16)
            xwmB = state.tile([128, MT, HM * BL], bf16)
            xtcA = state.tile([128, KT, CCOL], bf16)
            xtcB = state.tile([128, KT, CCOL], bf16)
            xtcC = state.tile([128, KT, CCOL], bf16)
            xtcD = state.tile([128, KT, CCOL], bf16)
            xwcA = state.tile([128, MT, CCOL], bf16)
            xwcB = state.tile([128, MT, CCOL], bf16)

            pre_ps = pre_ps_pool.tile([128, CCOL], f32)
            ps_g = ps_g_pool.tile([128, GB], f32)
            ps_f = ps_f_pool.tile([128, GB], f32)
            ps_i = ps_i_pool.tile([128, GB], f32)
            ps_o = ps_o_pool.tile([128, GB], f32)
            PS = {"g": ps_g, "f": ps_f, "i": ps_i, "o": ps_o}

            def xw_act(xwc, m, bias_src=None):
                nc.scalar.activation(
                    xwc[:, m, :], pre_ps[:], AF.Identity,
                    bias=(bias_src if bias_src is not None
                          else bias_sb)[:, m:m + 1],
                )

            # ---- Prologue: xw for cols [0, 2*MACRO*BL) the plain way ----
            for ch in range(4):
                xtc = xtcA if ch % 2 == 0 else xtcB
                xwc = xwcA if ch % 2 == 0 else xwcB
                cols = slice(ch * CCOL, (ch + 1) * CCOL)
                nc.gpsimd.dma_start(xtc[:], xT[:, :, cols])
                for m in range(MT):
                    for k in range(KT):
                        nc.tensor.matmul(
                            pre_ps[:],
                            W_sb[:, k, m * 128:(m + 1) * 128],
                            xtc[:, k, :],
                            start=(k == 0), stop=(k == KT - 1),
                        )
                    xw_act(xwc, m)
                nc.gpsimd.dma_start(xwT[:, :, cols], xwc[:])

            nc.vector.memset(hT_st[:], 0.0)
            nc.vector.memset(c_st[:], 0.0)
            nc.gpsimd.dma_start(xwmA[:], xwT[:, :, 0:HM * BL])
            # x for body 0's fused chunk 0 (cols 1 iteration ahead)
            nc.gpsimd.dma_start(
                xtcA[:], xT[:, :, MACRO * BL:MACRO * BL + CCOL]
            )

            def mm_group(gate, h_prev, xwm, slot):
                m0 = GATE_M[gate]
                dst = PS[gate]
                # inject xw via identity matmul (start=True), then
                # accumulate the 16 U matmuls on top.
                nc.tensor.matmul(
                    dst[:].rearrange("p (m b) -> p m b", m=4),
                    ident_sb[:],
                    xwm[:, m0:m0 + 4, slot * BL:(slot + 1) * BL],
                    start=True,
                    stop=False,
                    skip_group_check=True,
                )
                for j in range(4):
                    m = m0 + j
                    for k in range(KT):
                        nc.tensor.matmul(
                            dst[:, j * BL:(j + 1) * BL],
                            U_sb[:, k, m * 128:(m + 1) * 128],
                            h_prev[:, k, :],
                            start=False,
                            stop=(k == KT - 1),
                            skip_group_check=True,
                        )

            def macro_body(c0, unroll):
                assert unroll == 1
                # prefetch second half of this iteration's xw; load x for
                # this body's fused chunks 1-3
                nc.gpsimd.dma_start(
                    xwmB[:], xwT[:, :, bass.ds(c0 + HM * BL, HM * BL)]
                )
                nc.gpsimd.dma_start(
                    xtcB[:], xT[:, :, bass.ds(c0 + MACRO * BL + CCOL, CCOL)]
                )
                nc.gpsimd.dma_start(
                    xtcC[:], xT[:, :, bass.ds(c0 + MACRO * BL + 2 * CCOL, CCOL)]
                )
                nc.gpsimd.dma_start(
                    xtcD[:], xT[:, :, bass.ds(c0 + MACRO * BL + 3 * CCOL, CCOL)]
                )
                for s in range(MACRO):
                    if s == 0:
                        h_prev = hT_st[:].rearrange("p (k b) -> p k b", k=KT)
                    else:
                        h_prev = stage[:, :, (s - 1) * BL:s * BL]
                    xwm, slot = (xwmA, s) if s < HM else (xwmB, s - HM)
                    for gate in ("g", "f", "i", "o"):
                        mm_group(gate, h_prev, xwm, slot)
                    # fused precompute: one N=512 matmul per step
                    fch, fm, fk = s // CHUNK, (s % CHUNK) // KT, s % KT
                    fxtc = (xtcA, xtcB, xtcC, xtcD)[fch]
                    nc.tensor.matmul(
                        pre_ps[:],
                        W_sb[:, fk, fm * 128:(fm + 1) * 128],
                        fxtc[:, fk, :],
                        start=(fk == 0), stop=(fk == KT - 1),
                    )

                    tg = work.tile([128, GB], f32, tag="tg")
                    so = work.tile([128, GB], f32, tag="so")
                    cf = work.tile([128, GB], f32, tag="cf")
                    ig = work.tile([128, GB], f32, tag="ig")
                    a_f = a_f_pool.tile([128, GB], f32)
                    a_i = a_i_pool.tile([128, GB], f32)
                    thc = thc_pool.tile([128, GB], f32)

                    nc.scalar.activation(tg[:], ps_g[:], AF.Tanh)
                    nc.scalar.activation(a_f[:], ps_f[:], AF.Sigmoid)
                    nc.vector.tensor_mul(cf[:], a_f[:], c_st[:])
                    nc.scalar.activation(a_i[:], ps_i[:], AF.Sigmoid)
                    nc.vector.tensor_mul(ig[:], a_i[:], tg[:])
                    nc.vector.tensor_add(c_st[:], cf[:], ig[:])
                    nc.scalar.activation(so[:], ps_o[:], AF.Sigmoid)
                    nc.scalar.activation(thc[:], c_st[:], AF.Tanh)
                    if s == MACRO - 1:
                        hdst = hT_st[:].rearrange("p (k b) -> p k b", k=KT)
                    else:
                        hdst = stage[:, :, s * BL:(s + 1) * BL]
                    nc.vector.tensor_mul(
                        hdst,
                        so[:].rearrange("p (k b) -> p k b", k=KT),
                        thc[:].rearrange("p (k b) -> p k b", k=KT),
                    )
                    # fused precompute: bias + cast after each m finishes.
                    # bias is read through bias_step, refreshed by DVE right
                    # after hmul: a real dependency that stops the scheduler
                    # from running this 679ns ACT op before tanh(c).
                    if fk == KT - 1:
                        nc.vector.scalar_tensor_tensor(
                            bias_step[:], thc[:, 0:MT], 0.0, bias_sb[:],
                            op0=mybir.AluOpType.mult,
                            op1=mybir.AluOpType.add,
                        )
                        fxwc = xwcA if fch % 2 == 0 else xwcB
                        xw_act(fxwc, fm, bias_src=bias_step)
                        # store finished quarter of xw (4 m-tiles)
                        if fm % 4 == 3:
                            nc.gpsimd.dma_start(
                                xwT[:, fm - 3:fm + 1,
                                    bass.ds(c0 + MACRO * BL + fch * CCOL,
                                            CCOL)],
                                fxwc[:, fm - 3:fm + 1, :],
                            )
                    if s == MACRO - 1:
                        nc.vector.tensor_copy(
                            stage[:, :, s * BL:(s + 1) * BL],
                            hT_st[:].rearrange("p (k b) -> p k b", k=KT),
                        )
                    if s % (MACRO // 4) == MACRO // 4 - 1:
                        # stage out finished quarter of h
                        q0 = (s + 1 - MACRO // 4) * BL
                        nc.gpsimd.dma_start(
                            hT_out[:, :, bass.ds(c0 + q0, MACRO // 4 * BL)],
                            stage[:, :, q0:q0 + MACRO // 4 * BL],
                        )
                    if s == HM + 7:
                        # prefetch next iteration's first-half xw
                        nc.gpsimd.dma_start(
                            xwmA[:],
                            xwT[:, :, bass.ds(c0 + MACRO * BL, HM * BL)],
                        )
                    if s == HM + 15:
                        # x for next body's fused chunk 0
                        nc.gpsimd.dma_start(
                            xtcA[:],
                            xT[:, :, bass.ds(c0 + 2 * MACRO * BL, CCOL)],
                        )

            tc.For_i_unrolled_general(
                start=0, end=C, step=MACRO * BL,
                unrollable_body=macro_body, max_unroll=1,
                hint_engines=(mybir.EngineType.PE,),
            )
    nc.finalize()
    return nc


def kernel(x, W, U, bias):
    import concourse.bass as bass
    import concourse.bacc as bacc
    import concourse.tile as tile
    import concourse.mybir as mybir
    from concourse.bass_utils import run_bass_kernel_spmd

    x = np.asarray(x, np.float32)
    W = np.asarray(W, np.float32)
    U = np.asarray(U, np.float32)
    bias = np.asarray(bias, np.float32)

    nc = build(bacc.Bacc("TRN2", target_bir_lowering=False, num_devices=NC), bass, tile, mybir)

    Wt = np.ascontiguousarray(W.reshape(KT, 128, G4).transpose(1, 0, 2)).astype(ml_dtypes.bfloat16)
    Ut = np.ascontiguousarray(
        U.reshape(KT, 128, G4).transpose(1, 0, 2)
    ).astype(ml_dtypes.bfloat16)
    bt = np.ascontiguousarray(bias.reshape(MT, 128).T)

    in_maps = []
    for i in range(NC):
        xl = x[i * BL:(i + 1) * BL]                     # [8, 1024, 512]
        xTl = np.ascontiguousarray(
            xl.transpose(2, 1, 0).reshape(KT, 128, C)   # [512, T, 8]->[4,128,C]
        ).transpose(1, 0, 2)                            # [128, 4, C]
        xTl = np.concatenate(
            [xTl, np.zeros((128, KT, XPAD), xTl.dtype)], axis=2
        )
        in_maps.append({
            "xT": np.ascontiguousarray(xTl).astype(ml_dtypes.bfloat16),
            "W": Wt, "U": Ut, "biasT": bt,
            "ident": np.eye(128, dtype=ml_dtypes.bfloat16),
        })

    import os
    trace = bool(os.environ.get("LSTM_TRACE"))
    res = run_bass_kernel_spmd(
        nc, in_maps, core_ids=list(range(NC)), trace=trace
    )
    if trace and res.exec_time_ns is not None:
        print(f"HW exec time: {res.exec_time_ns} ns")
        print("trace:", (res.instructions_and_trace or (None, None))[1])
    out = np.empty((B, T, H), np.float32)
    for i in range(NC):
        ho = np.asarray(res.results[i]["hT_out"]).astype(np.float32)  # [128, 4, C]
        out[i * BL:(i + 1) * BL] = (
            ho.reshape(128, KT, T, BL).transpose(3, 2, 1, 0).reshape(BL, T, H)
        )
    return out


# revision 30
# speedup vs baseline: 1.1262x; 1.0273x over previous
"""Trainium2 Bass kernel for CustomLSTM: B=64, T=1024, I=H=512.

Sharding: data-parallel over batch, 8 sequences per core on 8 cores.
Everything on-device lives in TRANSPOSED layout (hidden/gate dim on SBUF
partitions, batch on the free dim) so the per-step elementwise chain runs on
all 128 lanes and h^T feeds the next step's matmul directly, zero transposes.

The x@W+bias precompute is FUSED into the recurrence loop: each 128-step
macro iteration computes the xw needed two iterations later, using one
N=512 matmul per step and one bias-activation per 4 steps in the PE/ACT
idle slots of the step; a short prologue covers the first two iterations.

Per step: per-gate PSUM tiles get xw injected by an identity matmul
(start=True), then the 16 U matmuls accumulate on top (start=False), so
each sigmoid/tanh reads its PSUM tile directly after its gate's matmuls.
Gate order g,f,i,o overlaps the elementwise chain with the PE phase; h is
written as bf16 straight into the staging tile, which is also the h state
consumed by the next step's matmuls.
"""

import numpy as np
import ml_dtypes

B, T, I, H = 64, 1024, 512, 512
NC = 8            # cores
BL = B // NC      # 8 sequences per core
G4 = 4 * H        # 2048 gate dim
KT = I // 128     # 4 contraction tiles
MT = G4 // 128    # 16 gate m-tiles
C = T * BL        # 8192 columns, col = t*8 + b
MACRO = 256       # timesteps per For_i iteration
HM = MACRO // 2   # half-macro (xwm double-buffer granularity)
CHUNK = 64        # timesteps per precompute chunk (512 columns)
CCOL = CHUNK * BL  # 512 cols per chunk
XPAD = 3072       # x pad: last body's fused reads + prefetches run ahead
WPAD = 2048       # xwT pad: fused precompute writes 1 iteration ahead

# m-tile gate map in W/U column order: i: 0-3, f: 4-7, g: 8-11, o: 12-15
GATE_M = {"i": 0, "f": 4, "g": 8, "o": 12}


def build(nc, bass, tile, mybir):
    f32, bf16 = mybir.dt.float32, mybir.dt.bfloat16
    AF = mybir.ActivationFunctionType

    xT = nc.dram_tensor("xT", [128, KT, C + XPAD], bf16, kind="ExternalInput")
    W = nc.dram_tensor("W", [128, KT, G4], bf16, kind="ExternalInput")
    U = nc.dram_tensor("U", [128, KT, G4], bf16, kind="ExternalInput")
    biasT = nc.dram_tensor("biasT", [128, MT], f32, kind="ExternalInput")
    ident = nc.dram_tensor("ident", [128, 128], bf16, kind="ExternalInput")
    hT_out = nc.dram_tensor("hT_out", [128, KT, C], bf16, kind="ExternalOutput")

    with tile.TileContext(nc) as tc:
        with (
            tc.tile_pool(name="const", bufs=1) as const,
            tc.tile_pool(name="state", bufs=1) as state,
            tc.tile_pool(name="work", bufs=2) as work,
            tc.tile_pool(name="dram", bufs=1, space="DRAM") as dram,
            tc.tile_pool(name="pre_ps", bufs=1, space="PSUM") as pre_ps_pool,
        ):
            W_sb = const.tile([128, KT, G4], bf16)
            U_sb = const.tile([128, KT, G4], bf16)
            bias_sb = const.tile([128, MT], f32)
            ident_sb = const.tile([128, 128], bf16)
            nc.gpsimd.dma_start(W_sb[:], W[:])
            nc.gpsimd.dma_start(U_sb[:], U[:])
            nc.gpsimd.dma_start(bias_sb[:], biasT[:])
            nc.gpsimd.dma_start(ident_sb[:], ident[:])

            # padded: fused precompute writes 2 iterations ahead
            xwT = dram.tile([128, MT, C + WPAD], bf16)

            GB = 4 * BL  # 32 cols per gate tile
            bias_step = state.tile([128, MT], f32)
            hT_st = state.tile([128, KT * BL], bf16)
            c_st = state.tile([128, KT * BL], f32)
            stage = state.tile([128, KT, MACRO * BL], bf16)
            xwmA = state.tile([128, MT, HM * BL], bf16)
            xwmB = state.tile([128, MT, HM * BL], bf16)
            xtcA = state.tile([128, KT, CCOL], bf16)
            xtcB = state.tile([128, KT, CCOL], bf16)
            xtcC = state.tile([128, KT, CCOL], bf16)
            xtcD = state.tile([128, KT, CCOL], bf16)
            xwcA = state.tile([128, MT, CCOL], bf16)
            xwcB = state.tile([128, MT, CCOL], bf16)

            pre_ps = pre_ps_pool.tile([128, CCOL], f32)

            def xw_act(xwc, m, bias_src=None, src=None):
                nc.scalar.activation(
                    xwc[:, m, :], (src if src is not None else pre_ps)[:],
                    AF.Identity,
                    bias=(bias_src if bias_src is not None
                          else bias_sb)[:, m:m + 1],
                )

            # ---- Prologue: xw for cols [0, MACRO*BL), double-buffered
            # PSUM (second bank scoped, released before phase-2 pools) ----
            with tc.tile_pool(name="pre2", bufs=1, space="PSUM") as pre2_pool:
                pre2 = pre2_pool.tile([128, CCOL], f32)
                for ch in range(4):
                    xtc = xtcA if ch % 2 == 0 else xtcB
                    xwc = xwcA if ch % 2 == 0 else xwcB
                    cols = slice(ch * CCOL, (ch + 1) * CCOL)
                    nc.gpsimd.dma_start(xtc[:], xT[:, :, cols])
                    for m in range(MT):
                        buf = pre_ps if m % 2 == 0 else pre2
                        for k in range(KT):
                            nc.tensor.matmul(
                                buf[:],
                                W_sb[:, k, m * 128:(m + 1) * 128],
                                xtc[:, k, :],
                                start=(k == 0), stop=(k == KT - 1),
                            )
                        xw_act(xwc, m, src=buf)
                    nc.gpsimd.dma_start(xwT[:, :, cols], xwc[:])

            import contextlib
            ps_stack = contextlib.ExitStack()
            ps_g_pool = ps_stack.enter_context(
                tc.tile_pool(name="ps_g", bufs=1, space="PSUM"))
            ps_f_pool = ps_stack.enter_context(
                tc.tile_pool(name="ps_f", bufs=1, space="PSUM"))
            ps_i_pool = ps_stack.enter_context(
                tc.tile_pool(name="ps_i", bufs=1, space="PSUM"))
            ps_o_pool = ps_stack.enter_context(
                tc.tile_pool(name="ps_o", bufs=1, space="PSUM"))
            a_f_pool = ps_stack.enter_context(
                tc.tile_pool(name="a_f", bufs=1, space="PSUM"))
            a_i_pool = ps_stack.enter_context(
                tc.tile_pool(name="a_i", bufs=1, space="PSUM"))
            thc_pool = ps_stack.enter_context(
                tc.tile_pool(name="thc", bufs=1, space="PSUM"))
            ps_g = ps_g_pool.tile([128, GB], f32)
            ps_f = ps_f_pool.tile([128, GB], f32)
            ps_i = ps_i_pool.tile([128, GB], f32)
            ps_o = ps_o_pool.tile([128, GB], f32)
            PS = {"g": ps_g, "f": ps_f, "i": ps_i, "o": ps_o}

            nc.vector.memset(hT_st[:], 0.0)
            nc.vector.memset(c_st[:], 0.0)
            nc.gpsimd.dma_start(xwmA[:], xwT[:, :, 0:HM * BL])
            # x for body 0's fused chunk 0 (cols 1 iteration ahead)
            nc.gpsimd.dma_start(
                xtcA[:], xT[:, :, MACRO * BL:MACRO * BL + CCOL]
            )

            def mm_group(gate, h_prev, xwm, slot):
                m0 = GATE_M[gate]
                dst = PS[gate]
                # inject xw via identity matmul (start=True), then
                # accumulate the 16 U matmuls on top.
                nc.tensor.matmul(
                    dst[:].rearrange("p (m b) -> p m b", m=4),
                    ident_sb[:],
                    xwm[:, m0:m0 + 4, slot * BL:(slot + 1) * BL],
                    start=True,
                    stop=False,
                    skip_group_check=True,
                )
                for j in range(4):
                    m = m0 + j
                    for k in range(KT):
                        nc.tensor.matmul(
                            dst[:, j * BL:(j + 1) * BL],
                            U_sb[:, k, m * 128:(m + 1) * 128],
                            h_prev[:, k, :],
                            start=False,
                            stop=(k == KT - 1),
                            skip_group_check=True,
                        )

            def macro_body(c0, unroll):
                assert unroll == 1
                # prefetch second half of this iteration's xw; load x for
                # this body's fused chunks 1-3
                nc.gpsimd.dma_start(
                    xwmB[:], xwT[:, :, bass.ds(c0 + HM * BL, HM * BL)]
                )
                nc.gpsimd.dma_start(
                    xtcB[:], xT[:, :, bass.ds(c0 + MACRO * BL + CCOL, CCOL)]
                )
                nc.gpsimd.dma_start(
                    xtcC[:], xT[:, :, bass.ds(c0 + MACRO * BL + 2 * CCOL, CCOL)]
                )
                nc.gpsimd.dma_start(
                    xtcD[:], xT[:, :, bass.ds(c0 + MACRO * BL + 3 * CCOL, CCOL)]
                )
                for s in range(MACRO):
                    if s == 0:
                        h_prev = hT_st[:].rearrange("p (k b) -> p k b", k=KT)
                    else:
                        h_prev = stage[:, :, (s - 1) * BL:s * BL]
                    xwm, slot = (xwmA, s) if s < HM else (xwmB, s - HM)
                    for gate in ("g", "f", "i", "o"):
                        mm_group(gate, h_prev, xwm, slot)
                    # fused precompute: one N=512 matmul per step
                    fch, fm, fk = s // CHUNK, (s % CHUNK) // KT, s % KT
                    fxtc = (xtcA, xtcB, xtcC, xtcD)[fch]
                    nc.tensor.matmul(
                        pre_ps[:],
                        W_sb[:, fk, fm * 128:(fm + 1) * 128],
                        fxtc[:, fk, :],
                        start=(fk == 0), stop=(fk == KT - 1),
                    )

                    tg = work.tile([128, GB], f32, tag="tg")
                    so = work.tile([128, GB], f32, tag="so")
                    cf = work.tile([128, GB], f32, tag="cf")
                    ig = work.tile([128, GB], f32, tag="ig")
                    a_f = a_f_pool.tile([128, GB], f32)
                    a_i = a_i_pool.tile([128, GB], f32)
                    thc = thc_pool.tile([128, GB], f32)

                    nc.scalar.activation(tg[:], ps_g[:], AF.Tanh)
                    nc.scalar.activation(a_f[:], ps_f[:], AF.Sigmoid)
                    nc.vector.tensor_mul(cf[:], a_f[:], c_st[:])
                    nc.scalar.activation(a_i[:], ps_i[:], AF.Sigmoid)
                    nc.vector.tensor_mul(ig[:], a_i[:], tg[:])
                    nc.vector.tensor_add(c_st[:], cf[:], ig[:])
                    nc.scalar.activation(so[:], ps_o[:], AF.Sigmoid)
                    nc.scalar.activation(thc[:], c_st[:], AF.Tanh)
                    if s == MACRO - 1:
                        hdst = hT_st[:].rearrange("p (k b) -> p k b", k=KT)
                    else:
                        hdst = stage[:, :, s * BL:(s + 1) * BL]
                    nc.vector.tensor_mul(
                        hdst,
                        so[:].rearrange("p (k b) -> p k b", k=KT),
                        thc[:].rearrange("p (k b) -> p k b", k=KT),
                    )
                    # fused precompute: bias + cast after each m finishes.
                    # bias is read through bias_step, refreshed by DVE right
                    # after hmul: a real dependency that stops the scheduler
                    # from running this 679ns ACT op before tanh(c).
                    if fk == KT - 1:
                        nc.vector.scalar_tensor_tensor(
                            bias_step[:], thc[:, 0:MT], 0.0, bias_sb[:],
                            op0=mybir.AluOpType.mult,
                            op1=mybir.AluOpType.add,
                        )
                        fxwc = xwcA if fch % 2 == 0 else xwcB
                        xw_act(fxwc, fm, bias_src=bias_step)
                        # store finished quarter of xw (4 m-tiles)
                        if fm % 4 == 3:
                            nc.gpsimd.dma_start(
                                xwT[:, fm - 3:fm + 1,
                                    bass.ds(c0 + MACRO * BL + fch * CCOL,
                                            CCOL)],
                                fxwc[:, fm - 3:fm + 1, :],
                            )
                    if s == MACRO - 1:
                        nc.vector.tensor_copy(
                            stage[:, :, s * BL:(s + 1) * BL],
                            hT_st[:].rearrange("p (k b) -> p k b", k=KT),
                        )
                    if s % (MACRO // 4) == MACRO // 4 - 1:
                        # stage out finished quarter of h
                        q0 = (s + 1 - MACRO // 4) * BL
                        nc.gpsimd.dma_start(
                            hT_out[:, :, bass.ds(c0 + q0, MACRO // 4 * BL)],
                            stage[:, :, q0:q0 + MACRO // 4 * BL],
                        )
                    if s == HM + 7:
                        # prefetch next iteration's first-half xw
                        nc.gpsimd.dma_start(
                            xwmA[:],
                            xwT[:, :, bass.ds(c0 + MACRO * BL, HM * BL)],
                        )
                    if s == HM + 15:
                        # x for next body's fused chunk 0
                        nc.gpsimd.dma_start(
                            xtcA[:],
                            xT[:, :, bass.ds(c0 + 2 * MACRO * BL, CCOL)],
                        )

            tc.For_i_unrolled_general(
                start=0, end=C, step=MACRO * BL,
                unrollable_body=macro_body, max_unroll=1,
                hint_engines=(mybir.EngineType.PE,),
            )
            ps_stack.close()
    nc.finalize()
    return nc


def kernel(x, W, U, bias):
    import concourse.bass as bass
    import concourse.bacc as bacc
    import concourse.tile as tile
    import concourse.mybir as mybir
    from concourse.bass_utils import run_bass_kernel_spmd

    x = np.asarray(x, np.float32)
    W = np.asarray(W, np.float32)
    U = np.asarray(U, np.float32)
    bias = np.asarray(bias, np.float32)

    nc = build(bacc.Bacc("TRN2", target_bir_lowering=False, num_devices=NC), bass, tile, mybir)

    Wt = np.ascontiguousarray(W.reshape(KT, 128, G4).transpose(1, 0, 2)).astype(ml_dtypes.bfloat16)
    Ut = np.ascontiguousarray(
        U.reshape(KT, 128, G4).transpose(1, 0, 2)
    ).astype(ml_dtypes.bfloat16)
    bt = np.ascontiguousarray(bias.reshape(MT, 128).T)

    in_maps = []
    for i in range(NC):
        xl = x[i * BL:(i + 1) * BL]                     # [8, 1024, 512]
        xTl = np.ascontiguousarray(
            xl.transpose(2, 1, 0).reshape(KT, 128, C)   # [512, T, 8]->[4,128,C]
        ).transpose(1, 0, 2)                            # [128, 4, C]
        xTl = np.concatenate(
            [xTl, np.zeros((128, KT, XPAD), xTl.dtype)], axis=2
        )
        in_maps.append({
            "xT": np.ascontiguousarray(xTl).astype(ml_dtypes.bfloat16),
            "W": Wt, "U": Ut, "biasT": bt,
            "ident": np.eye(128, dtype=ml_dtypes.bfloat16),
        })

    import os
    trace = bool(os.environ.get("LSTM_TRACE"))
    res = run_bass_kernel_spmd(
        nc, in_maps, core_ids=list(range(NC)), trace=trace
    )
    if trace and res.exec_time_ns is not None:
        print(f"HW exec time: {res.exec_time_ns} ns")
        print("trace:", (res.instructions_and_trace or (None, None))[1])
    out = np.empty((B, T, H), np.float32)
    for i in range(NC):
        ho = np.asarray(res.results[i]["hT_out"]).astype(np.float32)  # [128, 4, C]
        out[i * BL:(i + 1) * BL] = (
            ho.reshape(128, KT, T, BL).transpose(3, 2, 1, 0).reshape(BL, T, H)
        )
    return out


# revision 33
# speedup vs baseline: 1.1296x; 1.0029x over previous
"""Trainium2 Bass kernel for CustomLSTM: B=64, T=1024, I=H=512.

Sharding: data-parallel over batch, 8 sequences per core on 8 cores.
Everything on-device lives in TRANSPOSED layout (hidden/gate dim on SBUF
partitions, batch on the free dim) so the per-step elementwise chain runs on
all 128 lanes and h^T feeds the next step's matmul directly, zero transposes.

The x@W+bias precompute is FUSED into the recurrence loop: each 128-step
macro iteration computes the xw needed two iterations later, using one
N=512 matmul per step and one bias-activation per 4 steps in the PE/ACT
idle slots of the step; a short prologue covers the first two iterations.

Per step: per-gate PSUM tiles get xw injected by an identity matmul
(start=True), then the 16 U matmuls accumulate on top (start=False), so
each sigmoid/tanh reads its PSUM tile directly after its gate's matmuls.
Gate order g,f,i,o overlaps the elementwise chain with the PE phase; h is
written as bf16 straight into the staging tile, which is also the h state
consumed by the next step's matmuls.
"""

import numpy as np
import ml_dtypes

B, T, I, H = 64, 1024, 512, 512
NC = 8            # cores
BL = B // NC      # 8 sequences per core
G4 = 4 * H        # 2048 gate dim
KT = I // 128     # 4 contraction tiles
MT = G4 // 128    # 16 gate m-tiles
C = T * BL        # 8192 columns, col = t*8 + b
MACRO = 256       # timesteps per For_i iteration
HM = MACRO // 2   # half-macro (xwm double-buffer granularity)
CHUNK = 64        # timesteps per precompute chunk (512 columns)
CCOL = CHUNK * BL  # 512 cols per chunk
XPAD = 3072       # x pad: last body's fused reads + prefetches run ahead
WPAD = 2048       # xwT pad: fused precompute writes 1 iteration ahead

# m-tile gate map in W/U column order: i: 0-3, f: 4-7, g: 8-11, o: 12-15
GATE_M = {"i": 0, "f": 4, "g": 8, "o": 12}


def build(nc, bass, tile, mybir):
    f32, bf16 = mybir.dt.float32, mybir.dt.bfloat16
    AF = mybir.ActivationFunctionType

    xT = nc.dram_tensor("xT", [128, KT, C + XPAD], bf16, kind="ExternalInput")
    W = nc.dram_tensor("W", [128, KT, G4], bf16, kind="ExternalInput")
    U = nc.dram_tensor("U", [128, KT, G4], bf16, kind="ExternalInput")
    biasT = nc.dram_tensor("biasT", [128, MT], f32, kind="ExternalInput")
    ident = nc.dram_tensor("ident", [128, 128], bf16, kind="ExternalInput")
    hT_out = nc.dram_tensor("hT_out", [128, KT, C], bf16, kind="ExternalOutput")

    with tile.TileContext(nc) as tc:
        with (
            tc.tile_pool(name="const", bufs=1) as const,
            tc.tile_pool(name="state", bufs=1) as state,
            tc.tile_pool(name="work", bufs=2) as work,
            tc.tile_pool(name="dram", bufs=1, space="DRAM") as dram,
            tc.tile_pool(name="pre_ps", bufs=1, space="PSUM") as pre_ps_pool,
        ):
            W_sb = const.tile([128, KT, G4], bf16)
            U_sb = const.tile([128, KT, G4], bf16)
            bias_sb = const.tile([128, MT], f32)
            ident_sb = const.tile([128, 128], bf16)
            # Order matters on the gpsimd DMA queue: the prologue's first
            # matmul needs x chunk 0 + W only; U isn't needed until body 0.
            nc.gpsimd.dma_start(W_sb[:], W[:])
            nc.gpsimd.dma_start(bias_sb[:], biasT[:])
            nc.gpsimd.dma_start(ident_sb[:], ident[:])

            # padded: fused precompute writes 2 iterations ahead
            xwT = dram.tile([128, MT, C + WPAD], bf16)

            GB = 4 * BL  # 32 cols per gate tile
            bias_step = state.tile([128, MT], f32)
            hT_st = state.tile([128, KT * BL], bf16)
            c_st = state.tile([128, KT * BL], f32)
            stage = state.tile([128, KT, MACRO * BL], bf16)
            xwmA = state.tile([128, MT, HM * BL], bf16)
            xwmB = state.tile([128, MT, HM * BL], bf16)
            xtcA = state.tile([128, KT, CCOL], bf16)
            xtcB = state.tile([128, KT, CCOL], bf16)
            xtcC = state.tile([128, KT, CCOL], bf16)
            xtcD = state.tile([128, KT, CCOL], bf16)
            xwcA = state.tile([128, MT, CCOL], bf16)
            xwcB = state.tile([128, MT, CCOL], bf16)

            pre_ps = pre_ps_pool.tile([128, CCOL], f32)

            def xw_act(xwc, m, bias_src=None, src=None):
                nc.scalar.activation(
                    xwc[:, m, :], (src if src is not None else pre_ps)[:],
                    AF.Identity,
                    bias=(bias_src if bias_src is not None
                          else bias_sb)[:, m:m + 1],
                )

            # ---- Prologue: xw for cols [0, MACRO*BL), double-buffered
            # PSUM (second bank scoped, released before phase-2 pools) ----
            with tc.tile_pool(name="pre2", bufs=1, space="PSUM") as pre2_pool:
                pre2 = pre2_pool.tile([128, CCOL], f32)
                # load all 4 x chunks up front, then U (needed only later)
                for ch in range(4):
                    xtc = (xtcA, xtcB, xtcC, xtcD)[ch]
                    nc.gpsimd.dma_start(
                        xtc[:], xT[:, :, ch * CCOL:(ch + 1) * CCOL]
                    )
                nc.gpsimd.dma_start(U_sb[:], U[:])
                for ch in range(4):
                    xtc = (xtcA, xtcB, xtcC, xtcD)[ch]
                    xwc = xwcA if ch % 2 == 0 else xwcB
                    cols = slice(ch * CCOL, (ch + 1) * CCOL)
                    for m in range(MT):
                        buf = pre_ps if m % 2 == 0 else pre2
                        for k in range(KT):
                            nc.tensor.matmul(
                                buf[:],
                                W_sb[:, k, m * 128:(m + 1) * 128],
                                xtc[:, k, :],
                                start=(k == 0), stop=(k == KT - 1),
                            )
                        xw_act(xwc, m, src=buf)
                    nc.gpsimd.dma_start(xwT[:, :, cols], xwc[:])

            import contextlib
            ps_stack = contextlib.ExitStack()
            ps_g_pool = ps_stack.enter_context(
                tc.tile_pool(name="ps_g", bufs=1, space="PSUM"))
            ps_f_pool = ps_stack.enter_context(
                tc.tile_pool(name="ps_f", bufs=1, space="PSUM"))
            ps_i_pool = ps_stack.enter_context(
                tc.tile_pool(name="ps_i", bufs=1, space="PSUM"))
            ps_o_pool = ps_stack.enter_context(
                tc.tile_pool(name="ps_o", bufs=1, space="PSUM"))
            a_f_pool = ps_stack.enter_context(
                tc.tile_pool(name="a_f", bufs=1, space="PSUM"))
            a_i_pool = ps_stack.enter_context(
                tc.tile_pool(name="a_i", bufs=1, space="PSUM"))
            thc_pool = ps_stack.enter_context(
                tc.tile_pool(name="thc", bufs=1, space="PSUM"))
            ps_g = ps_g_pool.tile([128, GB], f32)
            ps_f = ps_f_pool.tile([128, GB], f32)
            ps_i = ps_i_pool.tile([128, GB], f32)
            ps_o = ps_o_pool.tile([128, GB], f32)
            PS = {"g": ps_g, "f": ps_f, "i": ps_i, "o": ps_o}

            nc.vector.memset(hT_st[:], 0.0)
            nc.vector.memset(c_st[:], 0.0)
            nc.gpsimd.dma_start(xwmA[:], xwT[:, :, 0:HM * BL])
            # x for body 0's fused chunk 0 (cols 1 iteration ahead)
            nc.gpsimd.dma_start(
                xtcA[:], xT[:, :, MACRO * BL:MACRO * BL + CCOL]
            )

            def mm_group(gate, h_prev, xwm, slot):
                m0 = GATE_M[gate]
                dst = PS[gate]
                # inject xw via identity matmul (start=True), then
                # accumulate the 16 U matmuls on top.
                nc.tensor.matmul(
                    dst[:].rearrange("p (m b) -> p m b", m=4),
                    ident_sb[:],
                    xwm[:, m0:m0 + 4, slot * BL:(slot + 1) * BL],
                    start=True,
                    stop=False,
                    skip_group_check=True,
                )
                for j in range(4):
                    m = m0 + j
                    for k in range(KT):
                        nc.tensor.matmul(
                            dst[:, j * BL:(j + 1) * BL],
                            U_sb[:, k, m * 128:(m + 1) * 128],
                            h_prev[:, k, :],
                            start=False,
                            stop=(k == KT - 1),
                            skip_group_check=True,
                        )

            def macro_body(c0, unroll):
                assert unroll == 1
                # prefetch second half of this iteration's xw; load x for
                # this body's fused chunks 1-3
                nc.gpsimd.dma_start(
                    xwmB[:], xwT[:, :, bass.ds(c0 + HM * BL, HM * BL)]
                )
                nc.gpsimd.dma_start(
                    xtcB[:], xT[:, :, bass.ds(c0 + MACRO * BL + CCOL, CCOL)]
                )
                nc.gpsimd.dma_start(
                    xtcC[:], xT[:, :, bass.ds(c0 + MACRO * BL + 2 * CCOL, CCOL)]
                )
                nc.gpsimd.dma_start(
                    xtcD[:], xT[:, :, bass.ds(c0 + MACRO * BL + 3 * CCOL, CCOL)]
                )
                for s in range(MACRO):
                    if s == 0:
                        h_prev = hT_st[:].rearrange("p (k b) -> p k b", k=KT)
                    else:
                        h_prev = stage[:, :, (s - 1) * BL:s * BL]
                    xwm, slot = (xwmA, s) if s < HM else (xwmB, s - HM)
                    for gate in ("g", "f", "i", "o"):
                        mm_group(gate, h_prev, xwm, slot)
                    # fused precompute: one N=512 matmul per step
                    fch, fm, fk = s // CHUNK, (s % CHUNK) // KT, s % KT
                    fxtc = (xtcA, xtcB, xtcC, xtcD)[fch]
                    nc.tensor.matmul(
                        pre_ps[:],
                        W_sb[:, fk, fm * 128:(fm + 1) * 128],
                        fxtc[:, fk, :],
                        start=(fk == 0), stop=(fk == KT - 1),
                    )

                    tg = work.tile([128, GB], f32, tag="tg")
                    so = work.tile([128, GB], f32, tag="so")
                    cf = work.tile([128, GB], f32, tag="cf")
                    ig = work.tile([128, GB], f32, tag="ig")
                    a_f = a_f_pool.tile([128, GB], f32)
                    a_i = a_i_pool.tile([128, GB], f32)
                    thc = thc_pool.tile([128, GB], f32)

                    nc.scalar.activation(tg[:], ps_g[:], AF.Tanh)
                    nc.scalar.activation(a_f[:], ps_f[:], AF.Sigmoid)
                    nc.vector.tensor_mul(cf[:], a_f[:], c_st[:])
                    nc.scalar.activation(a_i[:], ps_i[:], AF.Sigmoid)
                    nc.vector.tensor_mul(ig[:], a_i[:], tg[:])
                    nc.vector.tensor_add(c_st[:], cf[:], ig[:])
                    nc.scalar.activation(so[:], ps_o[:], AF.Sigmoid)
                    nc.scalar.activation(thc[:], c_st[:], AF.Tanh)
                    if s == MACRO - 1:
                        hdst = hT_st[:].rearrange("p (k b) -> p k b", k=KT)
                    else:
                        hdst = stage[:, :, s * BL:(s + 1) * BL]
                    nc.vector.tensor_mul(
                        hdst,
                        so[:].rearrange("p (k b) -> p k b", k=KT),
                        thc[:].rearrange("p (k b) -> p k b", k=KT),
                    )
                    # fused precompute: bias + cast after each m finishes.
                    # bias is read through bias_step, refreshed by DVE right
                    # after hmul: a real dependency that stops the scheduler
                    # from running this 679ns ACT op before tanh(c).
                    if fk == KT - 1:
                        nc.vector.scalar_tensor_tensor(
                            bias_step[:], thc[:, 0:MT], 0.0, bias_sb[:],
                            op0=mybir.AluOpType.mult,
                            op1=mybir.AluOpType.add,
                        )
                        fxwc = xwcA if fch % 2 == 0 else xwcB
                        xw_act(fxwc, fm, bias_src=bias_step)
                        # store finished quarter of xw (4 m-tiles)
                        if fm % 4 == 3:
                            nc.gpsimd.dma_start(
                                xwT[:, fm - 3:fm + 1,
                                    bass.ds(c0 + MACRO * BL + fch * CCOL,
                                            CCOL)],
                                fxwc[:, fm - 3:fm + 1, :],
                            )
                    if s == MACRO - 1:
                        nc.vector.tensor_copy(
                            stage[:, :, s * BL:(s + 1) * BL],
                            hT_st[:].rearrange("p (k b) -> p k b", k=KT),
                        )
                    if s % (MACRO // 4) == MACRO // 4 - 1:
                        # stage out finished quarter of h
                        q0 = (s + 1 - MACRO // 4) * BL
                        nc.gpsimd.dma_start(
                            hT_out[:, :, bass.ds(c0 + q0, MACRO // 4 * BL)],
                            stage[:, :, q0:q0 + MACRO // 4 * BL],
                        )
                    if s == HM + 7:
                        # prefetch next iteration's first-half xw
                        nc.gpsimd.dma_start(
                            xwmA[:],
                            xwT[:, :, bass.ds(c0 + MACRO * BL, HM * BL)],
                        )
                    if s == HM + 15:
                        # x for next body's fused chunk 0
                        nc.gpsimd.dma_start(
                            xtcA[:],
                            xT[:, :, bass.ds(c0 + 2 * MACRO * BL, CCOL)],
                        )

            tc.For_i_unrolled_general(
                start=0, end=C, step=MACRO * BL,
                unrollable_body=macro_body, max_unroll=1,
                hint_engines=(mybir.EngineType.PE,),
            )
            ps_stack.close()
    nc.finalize()
    return nc


def kernel(x, W, U, bias):
    import concourse.bass as bass
    import concourse.bacc as bacc
    import concourse.tile as tile
    import concourse.mybir as mybir
    from concourse.bass_utils import run_bass_kernel_spmd

    x = np.asarray(x, np.float32)
    W = np.asarray(W, np.float32)
    U = np.asarray(U, np.float32)
    bias = np.asarray(bias, np.float32)

    nc = build(bacc.Bacc("TRN2", target_bir_lowering=False, num_devices=NC), bass, tile, mybir)

    Wt = np.ascontiguousarray(W.reshape(KT, 128, G4).transpose(1, 0, 2)).astype(ml_dtypes.bfloat16)
    Ut = np.ascontiguousarray(
        U.reshape(KT, 128, G4).transpose(1, 0, 2)
    ).astype(ml_dtypes.bfloat16)
    bt = np.ascontiguousarray(bias.reshape(MT, 128).T)

    in_maps = []
    for i in range(NC):
        xl = x[i * BL:(i + 1) * BL]                     # [8, 1024, 512]
        xTl = np.ascontiguousarray(
            xl.transpose(2, 1, 0).reshape(KT, 128, C)   # [512, T, 8]->[4,128,C]
        ).transpose(1, 0, 2)                            # [128, 4, C]
        xTl = np.concatenate(
            [xTl, np.zeros((128, KT, XPAD), xTl.dtype)], axis=2
        )
        in_maps.append({
            "xT": np.ascontiguousarray(xTl).astype(ml_dtypes.bfloat16),
            "W": Wt, "U": Ut, "biasT": bt,
            "ident": np.eye(128, dtype=ml_dtypes.bfloat16),
        })

    import os
    trace = bool(os.environ.get("LSTM_TRACE"))
    res = run_bass_kernel_spmd(
        nc, in_maps, core_ids=list(range(NC)), trace=trace
    )
    if trace and res.exec_time_ns is not None:
        print(f"HW exec time: {res.exec_time_ns} ns")
        print("trace:", (res.instructions_and_trace or (None, None))[1])
    out = np.empty((B, T, H), np.float32)
    for i in range(NC):
        ho = np.asarray(res.results[i]["hT_out"]).astype(np.float32)  # [128, 4, C]
        out[i * BL:(i + 1) * BL] = (
            ho.reshape(128, KT, T, BL).transpose(3, 2, 1, 0).reshape(BL, T, H)
        )
    return out
